# revision 40
# baseline (speedup 1.0000x reference)
"""Distributed causal attention head for TRN2 (8 NeuronCores), v4.

Problem: B=4, S=4096, D=1024, H=64 fp32.
  q,k,v = x @ W{q,k,v}; scores = q k^T / sqrt(H); causal softmax; out = P v.

Design (collective-free, one SPMD-uniform NEFF, no barrier/AllGather/RS):
  - 4 batches x 2 cores per batch. Each core receives the FULL batch x^T
    (bf16, host-pretransposed) and projects Q^T for ALL 4096 queries
    itself; K/V only for the 2048 interleaved key rows it owns
    (128-chunk interleave keeps the causal work perfectly balanced).
  - SPMD uniformity: the host permutes x^T columns per core so the
    core's OWN key chunks sit at even 128-chunk positions. All extraction
    addresses are then identical across cores; causality differences are
    absorbed into per-core 0/1 mask DATA (queries are consistently in the
    permuted order; the host un-permutes the output).
  - Weights are host-reshaped so their DMAs use 1-2KB descriptors (the
    naive [D,128] layout produces 2k+ 256B descriptors that clog the
    queues ahead of the x strips).
  - The PE runs one continuous instruction stream (it only reaches its
    full 2.4GHz p-state when never blocked): Q|K projection quarters,
    V chunks, score chunk-pairs and AV pairs are interleaved so that by
    the time the PE reaches an instruction its inputs are long ready.
    Score pairs are "sprinkled" early between projection quarters to
    start the Scalar engine's exp stream (the 2nd-largest cost, ~40us)
    as soon as possible; exp results go to a big persistent SBUF P
    buffer (72KB/partition), and AV consumes P far behind the exp
    stream, so neither engine ever waits for the other.
  - V is augmented with a ones column so AV also emits the softmax
    denominator. Per-core partial (num^T | den) = [65, 4096] f32 goes
    straight to DRAM; the HOST adds the two partials of each pair,
    divides, and transposes. No on-device collective at all.
"""

import sys

sys.path.insert(0, "/opt/trn_rl_repo")

import numpy as np
import ml_dtypes

B, S, D, H = 4, 4096, 1024, 64
RPC = S // 2            # key rows owned per core
QB = 512                # query block width
NQB = S // QB           # 8 query blocks
NKC = RPC // 128        # 16 local key chunks
BF16 = ml_dtypes.bfloat16

_CACHE = {}


def _build():
    import concourse.bass as bass
    import concourse.mybir as mybir
    from concourse import bacc, tile
    from concourse.bass import ts

    f32 = mybir.dt.float32
    bf16 = mybir.dt.bfloat16
    Alu = mybir.AluOpType
    Act = mybir.ActivationFunctionType

    nc = bacc.Bacc(None, target_bir_lowering=False)

    xt_ext = nc.declare_dram_parameter("xt", [D, S], bf16, isOutput=False)
    # weights pre-shuffled on host: partition p holds all 8 dc-chunks
    wqk_ext = nc.declare_dram_parameter("wqk", [128, 8 * 128], bf16, isOutput=False)
    wv_ext = nc.declare_dram_parameter("wv", [128, 8 * H], bf16, isOutput=False)
    mask_ext = nc.declare_dram_parameter("mask", [128, 1024], bf16, isOutput=False)
    out_ext = nc.declare_dram_parameter("out", [H + 1, S], f32, isOutput=True)

    pairs = [(t, ip) for t in range(NQB) for ip in range(t + 1)]  # 36 chunk-pairs

    with tile.TileContext(nc) as tc:
        with tc.tile_pool(name="persist", bufs=1) as persist:
            # --- persistent SBUF tensors ---
            wqk_sb = persist.tile([128, 8, 128], bf16, tag="wqk")
            wv_sb = persist.tile([128, 8, H], bf16, tag="wv")
            mask_sb = persist.tile([128, 1024], bf16, tag="mask")
            qT = persist.tile([64, S], bf16, tag="qT")
            kT = persist.tile([64, RPC], bf16, tag="kT")
            v_all = persist.tile([128, NKC, H + 1], bf16, tag="v_all")
            p_sb = persist.tile([128, 36, 1024], bf16, tag="p")

            nc.vector.memset(v_all[:, :, H], 1.0)

            # weights+mask at the head of the gpsimd (SWDGE) queue so both
            # hardware queues start streaming x strips immediately
            nc.gpsimd.dma_start(out=wqk_sb[:], in_=wqk_ext[:])
            nc.gpsimd.dma_start(out=wv_sb[:], in_=wv_ext[:])
            nc.gpsimd.dma_start(out=mask_sb[:], in_=mask_ext[:])

            with tc.tile_pool(name="xt", bufs=1) as xt_pool:
                # x^T in two half-blocks with 4KB DMA lines, on the sync and
                # gpsimd queues ONLY: the scalar/ACT queue carries zero strip
                # DMAs, so the exp stream is never head-of-line blocked (a
                # queued DMA occupies its queue for the whole ~3us transfer).
                # block 0: sg0's columns only (1MiB) so the first score
                # pair reaches the Scalar engine earliest; then cols
                # 512:2048 (3KB lines) and cols 2048:4096 (4KB lines).
                q3 = [nc.sync, nc.gpsimd, nc.scalar]
                late_q = [nc.sync, nc.gpsimd]
                xb0 = xt_pool.tile([128, 8, 512], bf16, tag="xb0", name="xb0")
                for dc in range(8):
                    q3[dc % 3].dma_start(
                        out=xb0[:, dc, :], in_=xt_ext[ts(dc, 128), 0:512]
                    )
                xb1 = xt_pool.tile([128, 8, 1536], bf16, tag="xb1", name="xb1")
                for dc in range(8):
                    q3[(dc + 1) % 3].dma_start(
                        out=xb1[:, dc, :], in_=xt_ext[ts(dc, 128), 512:2048]
                    )
                xb2 = xt_pool.tile([128, 8, 2048], bf16, tag="xt", name="xb2")
                for dc in range(8):
                    late_q[dc % 2].dma_start(
                        out=xb2[:, dc, :], in_=xt_ext[ts(dc, 128), 2048:4096]
                    )

                def xt_cols(c0, w):
                    # map global column range -> (tile, local offset)
                    if c0 + w <= 512:
                        return xb0, c0
                    if c0 >= 2048:
                        return xb2, c0 - 2048
                    return xb1, c0 - 512

                state = {"st": 0}

                def emit_st_pair(j):
                    t, ip = pairs[j]
                    st2 = st_pool.tile([128, 1024], f32, tag="st", name=f"st{j}")
                    nc.tensor.matmul(
                        st2[:, 0:512],
                        lhsT=kT[:, 256 * ip : 256 * ip + 128],
                        rhs=qT[:, ts(t, QB)],
                        start=True,
                        stop=True,
                        skip_group_check=True,
                    )
                    nc.tensor.matmul(
                        st2[:, 512:1024],
                        lhsT=kT[:, 256 * ip + 128 : 256 * ip + 256],
                        rhs=qT[:, ts(t, QB)],
                        start=True,
                        stop=True,
                        skip_group_check=True,
                    )
                    nc.scalar.activation(p_sb[:, j, :], st2[:], Act.Exp, scale=0.125)
                    if ip == t:  # diagonal pair: multiplicative causal mask
                        nc.vector.tensor_tensor(
                            p_sb[:, j, :], p_sb[:, j, :], mask_sb[:], Alu.mult
                        )

                def emit_st_pairs(n, t_max):
                    while n > 0 and state["st"] < 36 and pairs[state["st"]][0] <= t_max:
                        emit_st_pair(state["st"])
                        state["st"] += 1
                        n -= 1

                av_tiles = {}
                pools = {}

                def emit_av_pair(j):
                    t, ip = pairs[j]
                    if ip == 0:
                        av_tiles[t] = pools["av"].tile(
                            [H + 1, QB], f32, tag="av", name=f"av{t}"
                        )
                    av = av_tiles[t]
                    nc.tensor.matmul(
                        av[:],
                        lhsT=v_all[:, 2 * ip, :],
                        rhs=p_sb[:, j, 0:512],
                        start=(ip == 0),
                        stop=False,
                        skip_group_check=True,
                    )
                    nc.tensor.matmul(
                        av[:],
                        lhsT=v_all[:, 2 * ip + 1, :],
                        rhs=p_sb[:, j, 512:1024],
                        start=False,
                        stop=(ip == t),
                        skip_group_check=True,
                    )
                    if ip == t:
                        o_sb = pools["o"].tile([H + 1, QB], f32, tag="o", name=f"o{t}")
                        nc.vector.tensor_copy(o_sb[:], av[:])
                        nc.sync.dma_start(out=out_ext[:, ts(t, QB)], in_=o_sb[:])

                with tc.tile_pool(name="st", bufs=2, space="PSUM") as st_pool:
                    av_state = {"av": 0}
                    vdone = {"v": -1}

                    def drain_avs(n):
                        while (
                            n > 0
                            and av_state["av"] < state["st"] - 2
                            and 2 * pairs[av_state["av"]][1] + 1 <= vdone["v"]
                        ):
                            emit_av_pair(av_state["av"])
                            av_state["av"] += 1
                            n -= 1

                    # --- Q|K projection per-sg (dc-outer, 8 mm per group) so
                    # the first score pair reaches the Scalar engine as early
                    # as possible; V chunks of each half fill the wait for
                    # half 1's strips ---
                    with (
                        tc.tile_pool(name="pj", bufs=2, space="PSUM") as pj_pool,
                        tc.tile_pool(name="pv", bufs=2, space="PSUM") as pv_pool,
                    ):
                        def emit_v_chunk(i):
                            vps = pv_pool.tile([128, H], f32, tag="v", name=f"v{i}")
                            xt_t, xt_o = xt_cols(256 * i, 128)
                            for dc in range(8):
                                nc.tensor.matmul(
                                    vps[:],
                                    lhsT=xt_t[:, dc, xt_o : xt_o + 128],
                                    rhs=wv_sb[:, dc, :],
                                    start=(dc == 0),
                                    stop=(dc == 7),
                                )
                            nc.vector.tensor_copy(v_all[:, i, 0:H], vps[:])
                            vdone["v"] = i

                        sprinkle = [1, 2, 3, 3, 4, 4, 3, 3]
                        for s in range(8):
                            qkp = pj_pool.tile(
                                [128, QB], f32, tag="qk", name=f"qk{s}"
                            )
                            xt_t, xt_o = xt_cols(512 * s, QB)
                            for dc in range(8):
                                nc.tensor.matmul(
                                    qkp[:],
                                    lhsT=wqk_sb[:, dc, :],
                                    rhs=xt_t[:, dc, xt_o : xt_o + QB],
                                    start=(dc == 0),
                                    stop=(dc == 7),
                                    skip_group_check=True,
                                )
                            nc.vector.tensor_copy(qT[:, ts(s, QB)], qkp[0:64, :])
                            nc.vector.tensor_copy(
                                kT[:, 256 * s : 256 * s + 128], qkp[64:128, 0:128]
                            )
                            nc.vector.tensor_copy(
                                kT[:, 256 * s + 128 : 256 * s + 256],
                                qkp[64:128, 256:384],
                            )
                            emit_st_pairs(sprinkle[s], s)
                            if s == 3:  # fill the wait for half 1's strips
                                for i in range(8):
                                    emit_v_chunk(i)
                                    emit_st_pairs(1, 3)
                            if s == 7:
                                for i in range(8, NKC):
                                    emit_v_chunk(i)
                                    emit_st_pairs(1, NQB - 1)

                    # --- remaining score pairs + all AV pairs ---
                    with (
                        tc.tile_pool(name="av", bufs=2, space="PSUM") as av_pool_,
                        tc.tile_pool(name="o", bufs=2) as o_pool_,
                    ):
                        pools["av"] = av_pool_
                        pools["o"] = o_pool_
                        while state["st"] < 36:
                            emit_st_pairs(1, NQB - 1)
                            drain_avs(6)
                        while av_state["av"] < 36:
                            emit_av_pair(av_state["av"])
                            av_state["av"] += 1

    nc.finalize()
    return nc


def _make_mask2(g: int) -> np.ndarray:
    """[128, 1024] multiplicative mask for the diagonal chunk pair of any
    query block t (t-independent thanks to the per-core permutation).

    Query columns are in permuted order: position pc in the block maps to
    global query chunk offsets delta = [g, 1-g, 2+g, 3-g] (relative to 4t).
    Left half masks own key chunk at global offset g; right half offset 2+g.
    """
    m = np.zeros((128, 1024), dtype=np.float32)
    delta = [g, 1 - g, 2 + g, 3 - g]
    kk = np.arange(128)[:, None]
    qq = np.arange(128)[None, :]
    for half, keyoff in ((0, g), (1, 2 + g)):
        for pc in range(4):
            keep = (128 * (delta[pc] - keyoff) + qq) >= kk
            m[:, half * 512 + pc * 128 : half * 512 + (pc + 1) * 128] = keep
    return m.astype(BF16)


def _swap_pairs(a: np.ndarray) -> np.ndarray:
    """Swap adjacent 128-column chunks (self-inverse permutation)."""
    n = a.shape[-1]
    return np.ascontiguousarray(
        a.reshape(a.shape[:-1] + (n // 256, 2, 128))[..., ::-1, :].reshape(a.shape)
    )


def _shard_inputs(input, Wq, Wk, Wv):
    wqk = np.concatenate([Wq, Wk], axis=1).astype(BF16)       # [1024, 128]
    wv = np.asarray(Wv).astype(BF16)                          # [1024, 64]
    # partition-major reshuffle so the SBUF load uses 1-2KB descriptors:
    # partition p holds [dc, col] for all 8 dc chunks
    wqk_r = np.ascontiguousarray(
        wqk.reshape(8, 128, 128).transpose(1, 0, 2).reshape(128, 8 * 128)
    )
    wv_r = np.ascontiguousarray(
        wv.reshape(8, 128, H).transpose(1, 0, 2).reshape(128, 8 * H)
    )
    masks = [_make_mask2(0), _make_mask2(1)]
    in_maps = []
    for b in range(B):
        xt = np.ascontiguousarray(np.asarray(input)[b].T).astype(BF16)
        xt_sw = _swap_pairs(xt)
        for g in range(2):
            in_maps.append(
                {
                    "xt": xt if g == 0 else xt_sw,
                    "wqk": wqk_r,
                    "wv": wv_r,
                    "mask": masks[g],
                }
            )
    return in_maps


def _unshard(results):
    out = np.empty((B, S, H), dtype=np.float32)
    for b in range(B):
        r0 = results[2 * b]["out"]                      # [65, S] natural order
        r1 = _swap_pairs(results[2 * b + 1]["out"])     # un-permute g=1
        m = r0 + r1
        out[b] = (m[:H] / m[H : H + 1]).T
    return out


def _run(inputs, trace=False):
    from concourse.bass_utils import run_bass_kernel_spmd

    if "nc" not in _CACHE:
        _CACHE["nc"] = _build()
    nc = _CACHE["nc"]
    in_maps = _shard_inputs(**inputs)
    res = run_bass_kernel_spmd(nc, in_maps, core_ids=list(range(8)), trace=trace)
    out = _unshard(res.results)
    return out, res


def kernel(**inputs) -> np.ndarray:
    out, _ = _run(inputs, trace=False)
    return out
